# revision 8
# baseline (speedup 1.0000x reference)
# Trainium2 Bass kernel for nn_Encoder_24575802868358 (char-CNN encoder).
#
# Computation (per word): char-embedding lookup -> depthwise grouped conv
# (30 groups x 30 multipliers, k=3, VALID) -> max-over-time pool -> concat
# with a GloVe embedding lookup.  Output [64, 128, 1000] f32.
#
# Strategy (data-parallel over the 8 cores; each core owns 1024 words):
#   * char gather: dma_gather(transpose=True) from a [102, 128]-padded bf16
#     char table produces X_T = [emb_dim -> partitions, positions -> free]
#     directly in SBUF.
#   * im2col: two SBUF->SBUF DMA shifted copies build XT3[30k+c, pos] =
#     X_T[c, pos+k] (k = 0 block is the gather output itself).
#   * conv as one K=90 matmul per (channel-tile, position-tile): the
#     block-diagonal fused weight W2[(k*30+c), o] = w[o, k] * [c == o//30]
#     turns the depthwise conv into dense matmuls; PSUM gets [ch, word, t].
#   * max-over-time: DVE tensor_reduce(max) over a strided [ch, 4, 8, 38]
#     view of PSUM; bias added afterwards (max(y)+b == max(y+b)).
#   * GloVe rows: indirect DMA gather (int32 indices), copied straight to
#     the output.
# Host side only reshapes/concatenates per-core outputs.

import numpy as np
import ml_dtypes

import concourse.bass as bass
import concourse.mybir as mybir
from concourse import bacc
from concourse.tile import TileContext
from concourse.bass_utils import run_bass_kernel_spmd

# ---------------------------------------------------------------- constants
B, S, WLEN = 64, 128, 40
CHAR_EMB = 30
N_FILT = 30
KSIZE = 3
WORD_EMB = 100
N_CHARS = 102
VOCAB = 400002
NCH = CHAR_EMB * N_FILT          # 900 conv output channels
TVALID = WLEN - KSIZE + 1        # 38 valid conv positions

NCORES = 8
WORDS = B * S                    # 8192
WPC = WORDS // NCORES            # 1024 words per core
NCHUNK = 4
CHUNK_W = WPC // NCHUNK          # 256 words per chunk
POS_C = CHUNK_W * WLEN           # 10240 positions per chunk
NI = 10368                       # gather idxs per chunk (81*128 >= POS_C+2)
IDX_COLS = NI // 16              # 648

CHT = [113, 113, 113, 113, 112, 112, 112, 112]   # channel tile sizes
CHOFF = np.concatenate([[0], np.cumsum(CHT)])    # offsets into 900

PSUM_FREE = 2048                 # 4 banks per psum tile
WORDS_PER_MM = 8                 # words per matmul -> N = 8*38 = 304
MM_N = WORDS_PER_MM * TVALID     # 304
MM_PER_PS = 4                    # matmuls per psum tile (4 x 512 slices)
WORDS_PER_PS = WORDS_PER_MM * MM_PER_PS          # 32
PS_GROUPS = CHUNK_W // WORDS_PER_PS              # 8

_CACHE = {}


# ---------------------------------------------------------------- program
def build_program():
    nc = bacc.Bacc("TRN2", target_bir_lowering=False)
    f32, bf16 = mybir.dt.float32, mybir.dt.bfloat16

    ctab = nc.declare_dram_parameter("ctab", [N_CHARS, 128], bf16, isOutput=False)
    w2 = nc.declare_dram_parameter("w2", [90, NCH], bf16, isOutput=False)
    biasp = nc.declare_dram_parameter("biasp", [128, 8], f32, isOutput=False)
    cidx = nc.declare_dram_parameter("cidx", [NCHUNK, 128, IDX_COLS], mybir.dt.int16,
                                     isOutput=False)
    gidx = nc.declare_dram_parameter("gidx", [128, WPC // 128], mybir.dt.int32,
                                     isOutput=False)
    glove = nc.declare_dram_parameter("glove", [VOCAB, WORD_EMB], f32, isOutput=False)
    pooled_d = nc.declare_dram_parameter("pooled", [NCH, WPC], f32, isOutput=True)
    gout_d = nc.declare_dram_parameter("gout", [WPC, WORD_EMB], f32, isOutput=True)

    # Register write must precede the gathers; keep it out of Tile scheduling.
    ni_reg = nc.gpsimd.to_reg(NI)

    with TileContext(nc) as tc:
        with (
            tc.tile_pool(name="const", bufs=1) as cpool,
            tc.tile_pool(name="work", bufs=2) as wpool,
            tc.tile_pool(name="small", bufs=3) as spool,
            tc.tile_pool(name="ps", bufs=2, space="PSUM") as pspool,
        ):
            w2_s = cpool.tile([90, NCH], bf16, name="w2_s")
            nc.sync.dma_start(out=w2_s[:], in_=w2[:])
            bias_s = cpool.tile([128, 8], f32, name="bias_s")
            nc.sync.dma_start(out=bias_s[:], in_=biasp[:])
            gidx_s = cpool.tile([128, WPC // 128], mybir.dt.int32, name="gidx_s")
            nc.sync.dma_start(out=gidx_s[:], in_=gidx[:])

            # GloVe gather: 8 blocks of 128 words.
            for j in range(WPC // 128):
                gv = spool.tile([128, WORD_EMB], f32, tag="gv", name="gv")
                nc.gpsimd.indirect_dma_start(
                    out=gv[:],
                    out_offset=None,
                    in_=glove[:],
                    in_offset=bass.IndirectOffsetOnAxis(ap=gidx_s[:, j:j + 1], axis=0),
                )
                nc.sync.dma_start(out=gout_d[j * 128:(j + 1) * 128, :], in_=gv[:])

            for c in range(NCHUNK):
                idx_t = wpool.tile([128, IDX_COLS], mybir.dt.int16, tag="idx",
                                   name="idx_t")
                nc.sync.dma_start(out=idx_t[:], in_=cidx[c])

                # gather -> partitions 0..29 = X_T rows (30..127 are zeros)
                xt3 = wpool.tile([128, NI], bf16, tag="xt3", name="xt3")
                nc.gpsimd.dma_gather(
                    out_ap=xt3[:].rearrange("p (a n) -> p a n", a=1),
                    in_ap=ctab[:],
                    idxs_ap=idx_t[:],
                    num_idxs=NI,
                    num_idxs_reg=ni_reg,
                    elem_size=128,
                    transpose=True,
                    # >64 descriptors per packet overflows the 16KB packet
                    # limit and kills the device — keep packets split.
                    single_packet=False,
                )
                # im2col shifted copies (partition-crossing => DMA)
                nc.sync.dma_start(out=xt3[30:60, 0:POS_C], in_=xt3[0:30, 1:POS_C + 1])
                nc.sync.dma_start(out=xt3[60:90, 0:POS_C], in_=xt3[0:30, 2:POS_C + 2])

                xw = xt3[0:90, 0:POS_C].rearrange("p (w t) -> p w t", t=WLEN)

                for m in range(8):
                    mo, msz = int(CHOFF[m]), CHT[m]
                    pooled_t = spool.tile([128, CHUNK_W], f32, tag="pooled",
                                          name="pooled_t")
                    for g in range(PS_GROUPS):
                        ps = pspool.tile([128, PSUM_FREE], f32, tag="ps", name="ps")
                        for j in range(MM_PER_PS):
                            w0 = g * WORDS_PER_PS + j * WORDS_PER_MM
                            nc.tensor.matmul(
                                out=ps[0:msz, 512 * j:512 * j + MM_N],
                                lhsT=w2_s[:, mo:mo + msz],
                                rhs=xw[:, w0:w0 + WORDS_PER_MM, 0:TVALID],
                                start=True,
                                stop=True,
                            )
                        rin = (
                            ps[0:msz, :]
                            .rearrange("p (b s) -> p b s", s=512)[:, :, 0:MM_N]
                            .rearrange("p b (w t) -> p b w t", t=TVALID)
                        )
                        nc.vector.reduce_max(
                            out=pooled_t[0:msz,
                                         g * WORDS_PER_PS:(g + 1) * WORDS_PER_PS],
                            in_=rin,
                            axis=mybir.AxisListType.X,
                        )
                    nc.vector.tensor_scalar_add(
                        out=pooled_t[0:msz, :],
                        in0=pooled_t[0:msz, :],
                        scalar1=bias_s[0:msz, m:m + 1],
                    )
                    nc.sync.dma_start(
                        out=pooled_d[mo:mo + msz, c * CHUNK_W:(c + 1) * CHUNK_W],
                        in_=pooled_t[0:msz, :],
                    )
    return nc


# ---------------------------------------------------------------- host prep
def _wrap_idx16(flat):
    """flat int array (len % 16 == 0) -> [128, len/16] int16 gather layout."""
    a = flat.astype(np.int16).reshape(-1, 16).T          # [16, S]
    return np.tile(a, (8, 1))                            # replicate to 128 parts


def _prep_shared(char_emb, conv_w, conv_b, glove):
    ctab = np.zeros((N_CHARS, 128), ml_dtypes.bfloat16)
    ctab[:, :CHAR_EMB] = char_emb.astype(ml_dtypes.bfloat16)

    w2 = np.zeros((90, NCH), np.float32)
    o = np.arange(NCH)
    g = o // N_FILT
    for k in range(KSIZE):
        w2[k * CHAR_EMB + g, o] = conv_w[o, 0, k]
    w2 = w2.astype(ml_dtypes.bfloat16)

    biasp = np.zeros((128, 8), np.float32)
    for m in range(8):
        biasp[:CHT[m], m] = conv_b[CHOFF[m]:CHOFF[m] + CHT[m]]

    glove = np.ascontiguousarray(glove.astype(np.float32))
    return ctab, w2, biasp, glove


def _prep_core(core, char_ids_flat_pad, word_ids_flat):
    cidx = np.empty((NCHUNK, 128, IDX_COLS), np.int16)
    base = core * WPC * WLEN
    for c in range(NCHUNK):
        seg = np.zeros(NI, np.int64)
        seg[:POS_C + 2] = char_ids_flat_pad[base + c * POS_C:
                                            base + c * POS_C + POS_C + 2]
        cidx[c] = _wrap_idx16(seg)
    wseg = word_ids_flat[core * WPC:(core + 1) * WPC].astype(np.int32)
    gidx = wseg.reshape(WPC // 128, 128).T.copy()        # [128, 8]
    return cidx, gidx


def kernel(char_ids, word_ids, char_emb, conv_w, conv_b, glove):
    char_ids = np.asarray(char_ids)
    word_ids = np.asarray(word_ids)
    char_emb = np.asarray(char_emb, np.float32)
    conv_w = np.asarray(conv_w, np.float32)
    conv_b = np.asarray(conv_b, np.float32)
    glove = np.asarray(glove, np.float32)

    if "nc" not in _CACHE:
        nc = build_program()
        if not nc.is_finalized():
            nc.finalize()   # Bacc compile passes (register alloc, lib loads)
        _CACHE["nc"] = nc
    nc = _CACHE["nc"]

    ctab, w2, biasp, glove_c = _prep_shared(char_emb, conv_w, conv_b, glove)

    ids_flat_pad = np.concatenate(
        [char_ids.reshape(-1), np.zeros(2, np.int64)]).astype(np.int64)
    word_ids_flat = word_ids.reshape(-1)

    in_maps = []
    for core in range(NCORES):
        cidx, gidx = _prep_core(core, ids_flat_pad, word_ids_flat)
        in_maps.append({
            "ctab": ctab,
            "w2": w2,
            "biasp": biasp,
            "cidx": cidx,
            "gidx": gidx,
            "glove": glove_c,
        })

    import os
    trace = bool(int(os.environ.get("BASS_KERNEL_TRACE", "0")))
    res = run_bass_kernel_spmd(nc, in_maps, core_ids=list(range(NCORES)),
                               trace=trace)
    _CACHE["last_result"] = res
    results = res.results

    out = np.empty((WORDS, NCH + WORD_EMB), np.float32)
    for core in range(NCORES):
        r = results[core]
        out[core * WPC:(core + 1) * WPC, :NCH] = r["pooled"].T
        out[core * WPC:(core + 1) * WPC, NCH:] = r["gout"]
    return out.reshape(B, S, NCH + WORD_EMB)


# revision 10
# speedup vs baseline: 1.4606x; 1.4606x over previous
# Trainium2 Bass kernel for nn_Encoder_24575802868358 (char-CNN encoder).
#
# Computation (per word): char-embedding lookup -> depthwise grouped conv
# (30 groups x 30 multipliers, k=3, VALID) -> max-over-time pool -> concat
# with a GloVe embedding lookup.  Output [64, 128, 1000] f32.
#
# Strategy (data-parallel over the 8 cores; each core owns 1024 words):
#   * char "gather" via one-hot matmul: char ids are broadcast-DMA'd across
#     102 partitions, DVE is_equal against a per-partition iota builds the
#     one-hot OHT[v, pos] (4x bf16 mode), and a PE matmul with the char
#     table as stationary weights produces X_T[c, pos] directly in the
#     transposed layout the conv needs.  (A per-position indirect gather
#     costs ~300us of GPSIMD descriptor generation - measured - while this
#     path is ~15us PE + ~11us DVE.)
#   * im2col: ScalarE evicts X_T from PSUM to SBUF (bf16), two SBUF->SBUF
#     DMA shifted copies build XT3[30k+c, pos] = X_T[c, pos+k].
#   * conv as one K=90 matmul per (channel-tile, 13-word position tile):
#     the block-diagonal fused weight W2[(k*30+c), o] = w[o,k] * [c==o//30]
#     turns the depthwise conv into dense matmuls; PSUM gets [ch, word, t].
#   * max-over-time: DVE tensor_reduce(max) over strided [ch, 2, 13, 38]
#     PSUM views; bias added on ScalarE afterwards (max(y)+b == max(y+b)).
#   * GloVe rows: indirect DMA gather (int32 indices) -> output.
# Host side only reshapes/concatenates per-core outputs.

import numpy as np
import ml_dtypes

import concourse.bass as bass
import concourse.mybir as mybir
from concourse import bacc
from concourse.tile import TileContext
from concourse.bass_utils import run_bass_kernel_spmd

# ---------------------------------------------------------------- constants
B, S, WLEN = 64, 128, 40
CHAR_EMB = 30
N_FILT = 30
KSIZE = 3
WORD_EMB = 100
N_CHARS = 102
VOCAB = 400002
NCH = CHAR_EMB * N_FILT          # 900 conv output channels
TVALID = WLEN - KSIZE + 1        # 38 valid conv positions

NCORES = 8
WORDS = B * S                    # 8192
WPC = WORDS // NCORES            # 1024 words per core
NCHUNK = 4
CHUNK_W = WPC // NCHUNK          # 256 words per chunk
POS_C = CHUNK_W * WLEN           # 10240 positions per chunk
NI = 10368                       # padded positions per chunk (>= POS_C+2)

CHT = [113, 113, 113, 113, 112, 112, 112, 112]   # channel tile sizes
CHOFF = np.concatenate([[0], np.cumsum(CHT)])    # offsets into 900

# conv matmuls: 19 x 13-word (N=494) + 1 x 9-word (N=342) per chunk/chtile,
# paired two per [128, 1024] PSUM tile (at free offsets 0 and 512).
MM_WORDS = [13] * 19 + [9]

_CACHE = {}


# ---------------------------------------------------------------- program
def build_program():
    nc = bacc.Bacc("TRN2", target_bir_lowering=False)
    f32, bf16 = mybir.dt.float32, mybir.dt.bfloat16

    ctab = nc.declare_dram_parameter("ctab", [N_CHARS, CHAR_EMB], bf16,
                                     isOutput=False)
    w2 = nc.declare_dram_parameter("w2", [90, NCH], bf16, isOutput=False)
    biasp = nc.declare_dram_parameter("biasp", [128, 8], f32, isOutput=False)
    iota = nc.declare_dram_parameter("iota", [128, 1], f32, isOutput=False)
    cids = nc.declare_dram_parameter("cids", [NCHUNK, NI], bf16, isOutput=False)
    gidx = nc.declare_dram_parameter("gidx", [128, WPC // 128], mybir.dt.int32,
                                     isOutput=False)
    glove = nc.declare_dram_parameter("glove", [VOCAB, WORD_EMB], f32,
                                     isOutput=False)
    pooled_d = nc.declare_dram_parameter("pooled", [NCH, WPC], f32, isOutput=True)
    gout_d = nc.declare_dram_parameter("gout", [WPC, WORD_EMB], f32, isOutput=True)

    with TileContext(nc) as tc:
        with (
            tc.tile_pool(name="const", bufs=1) as cpool,
            tc.tile_pool(name="work", bufs=2) as wpool,
            tc.tile_pool(name="small", bufs=3) as spool,
            tc.tile_pool(name="cps", bufs=3, space="PSUM") as cpspool,
            tc.tile_pool(name="xps", bufs=2, space="PSUM") as xpspool,
        ):
            w2_s = cpool.tile([90, NCH], bf16, name="w2_s")
            nc.sync.dma_start(out=w2_s[:], in_=w2[:])
            bias_s = cpool.tile([128, 8], f32, name="bias_s")
            nc.sync.dma_start(out=bias_s[:], in_=biasp[:])
            iota_s = cpool.tile([128, 1], f32, name="iota_s")
            nc.sync.dma_start(out=iota_s[:], in_=iota[:])
            ctab_s = cpool.tile([N_CHARS, CHAR_EMB], bf16, name="ctab_s")
            nc.sync.dma_start(out=ctab_s[:], in_=ctab[:])
            gidx_s = cpool.tile([128, WPC // 128], mybir.dt.int32, name="gidx_s")
            nc.sync.dma_start(out=gidx_s[:], in_=gidx[:])

            # GloVe gather: 8 blocks of 128 words.
            for j in range(WPC // 128):
                gv = spool.tile([128, WORD_EMB], f32, tag="gv", name="gv")
                nc.gpsimd.indirect_dma_start(
                    out=gv[:],
                    out_offset=None,
                    in_=glove[:],
                    in_offset=bass.IndirectOffsetOnAxis(ap=gidx_s[:, j:j + 1], axis=0),
                )
                nc.sync.dma_start(out=gout_d[j * 128:(j + 1) * 128, :], in_=gv[:])

            for c in range(NCHUNK):
                # ids broadcast across the 102 char-value partitions
                idsb = wpool.tile([N_CHARS, NI], bf16, tag="idsb", name="idsb")
                nc.sync.dma_start(out=idsb[:],
                                  in_=cids[c:c + 1, :].to_broadcast([N_CHARS, NI]))
                # one-hot: OHT[v, pos] = (ids[pos] == v)
                oht = wpool.tile([N_CHARS, NI], bf16, tag="oht", name="oht")
                nc.vector.tensor_scalar(out=oht[:], in0=idsb[:],
                                        scalar1=iota_s[0:N_CHARS, 0:1],
                                        scalar2=None,
                                        op0=mybir.AluOpType.is_equal)
                # X_T[c, pos] = char_emb[ids[pos], c] via PE + ScalarE evict
                xt3 = wpool.tile([128, NI], bf16, tag="xt3", name="xt3")
                for q in range(NI // 512 + 1):
                    q0 = 512 * q
                    qn = min(512, NI - q0)
                    if qn <= 0:
                        break
                    xps = xpspool.tile([CHAR_EMB, 512], f32, tag="xps", name="xps")
                    nc.tensor.matmul(out=xps[:, 0:qn],
                                     lhsT=ctab_s[:],
                                     rhs=oht[:, q0:q0 + qn],
                                     start=True, stop=True)
                    nc.scalar.copy(out=xt3[0:CHAR_EMB, q0:q0 + qn],
                                   in_=xps[:, 0:qn])
                # im2col shifted copies (partition-crossing => DMA)
                nc.sync.dma_start(out=xt3[30:60, 0:POS_C],
                                  in_=xt3[0:30, 1:POS_C + 1])
                nc.sync.dma_start(out=xt3[60:90, 0:POS_C],
                                  in_=xt3[0:30, 2:POS_C + 2])

                xw = xt3[0:90, 0:POS_C].rearrange("p (w t) -> p w t", t=WLEN)

                for m in range(8):
                    mo, msz = int(CHOFF[m]), CHT[m]
                    pooled_t = spool.tile([128, CHUNK_W], f32, tag="pooled",
                                          name="pooled_t")
                    w0 = 0
                    mm_i = 0
                    while mm_i < len(MM_WORDS):
                        ps = cpspool.tile([128, 1024], f32, tag="cps", name="ps")
                        nwl = []
                        for half in range(2):
                            if mm_i >= len(MM_WORDS):
                                break
                            nw = MM_WORDS[mm_i]
                            nc.tensor.matmul(
                                out=ps[0:msz, 512 * half:512 * half + nw * TVALID],
                                lhsT=w2_s[:, mo:mo + msz],
                                rhs=xw[:, w0 + sum(nwl):w0 + sum(nwl) + nw, 0:TVALID],
                                start=True, stop=True,
                            )
                            nwl.append(nw)
                            mm_i += 1
                        if len(nwl) == 2 and nwl[0] == nwl[1]:
                            rin = (
                                ps[0:msz, :]
                                .rearrange("p (b s) -> p b s", s=512)
                                [:, :, 0:nwl[0] * TVALID]
                                .rearrange("p b (w t) -> p b w t", t=TVALID)
                            )
                            nc.vector.reduce_max(
                                out=pooled_t[0:msz, w0:w0 + 2 * nwl[0]],
                                in_=rin, axis=mybir.AxisListType.X)
                        else:
                            off = 0
                            for half, nw in enumerate(nwl):
                                rin = (
                                    ps[0:msz, 512 * half:512 * half + nw * TVALID]
                                    .rearrange("p (w t) -> p w t", t=TVALID)
                                )
                                nc.vector.reduce_max(
                                    out=pooled_t[0:msz, w0 + off:w0 + off + nw],
                                    in_=rin, axis=mybir.AxisListType.X)
                                off += nw
                        w0 += sum(nwl)
                    # bias add on ScalarE (out = in*1 + bias[p])
                    nc.scalar.add(out=pooled_t[0:msz, :], in_=pooled_t[0:msz, :],
                                  add=bias_s[0:msz, m:m + 1])
                    nc.sync.dma_start(
                        out=pooled_d[mo:mo + msz, c * CHUNK_W:(c + 1) * CHUNK_W],
                        in_=pooled_t[0:msz, :],
                    )
    return nc


# ---------------------------------------------------------------- host prep
def _prep_shared(char_emb, conv_w, conv_b, glove):
    ctab = char_emb.astype(ml_dtypes.bfloat16)           # [102, 30]

    w2 = np.zeros((90, NCH), np.float32)
    o = np.arange(NCH)
    g = o // N_FILT
    for k in range(KSIZE):
        w2[k * CHAR_EMB + g, o] = conv_w[o, 0, k]
    w2 = w2.astype(ml_dtypes.bfloat16)

    biasp = np.zeros((128, 8), np.float32)
    for m in range(8):
        biasp[:CHT[m], m] = conv_b[CHOFF[m]:CHOFF[m] + CHT[m]]

    iota = np.arange(128, dtype=np.float32).reshape(128, 1)
    glove = np.ascontiguousarray(glove.astype(np.float32))
    return ctab, w2, biasp, iota, glove


def _prep_core(core, char_ids_flat_pad, word_ids_flat):
    cids = np.zeros((NCHUNK, NI), np.float32)
    base = core * WPC * WLEN
    for c in range(NCHUNK):
        cids[c, :POS_C + 2] = char_ids_flat_pad[base + c * POS_C:
                                                base + c * POS_C + POS_C + 2]
    cids = cids.astype(ml_dtypes.bfloat16)
    wseg = word_ids_flat[core * WPC:(core + 1) * WPC].astype(np.int32)
    gidx = wseg.reshape(WPC // 128, 128).T.copy()        # [128, 8]
    return cids, gidx


def kernel(char_ids, word_ids, char_emb, conv_w, conv_b, glove):
    char_ids = np.asarray(char_ids)
    word_ids = np.asarray(word_ids)
    char_emb = np.asarray(char_emb, np.float32)
    conv_w = np.asarray(conv_w, np.float32)
    conv_b = np.asarray(conv_b, np.float32)
    glove = np.asarray(glove, np.float32)

    if "nc" not in _CACHE:
        nc = build_program()
        if not nc.is_finalized():
            nc.finalize()   # Bacc compile passes (register alloc, lib loads)
        _CACHE["nc"] = nc
    nc = _CACHE["nc"]

    ctab, w2, biasp, iota, glove_c = _prep_shared(char_emb, conv_w, conv_b, glove)

    ids_flat_pad = np.concatenate(
        [char_ids.reshape(-1), np.zeros(2, np.int64)]).astype(np.int64)
    word_ids_flat = word_ids.reshape(-1)

    in_maps = []
    for core in range(NCORES):
        cids, gidx = _prep_core(core, ids_flat_pad, word_ids_flat)
        in_maps.append({
            "ctab": ctab,
            "w2": w2,
            "biasp": biasp,
            "iota": iota,
            "cids": cids,
            "gidx": gidx,
            "glove": glove_c,
        })

    import os
    trace = bool(int(os.environ.get("BASS_KERNEL_TRACE", "0")))
    res = run_bass_kernel_spmd(nc, in_maps, core_ids=list(range(NCORES)),
                               trace=trace)
    _CACHE["last_result"] = res
    results = res.results

    out = np.empty((WORDS, NCH + WORD_EMB), np.float32)
    for core in range(NCORES):
        r = results[core]
        out[core * WPC:(core + 1) * WPC, :NCH] = r["pooled"].T
        out[core * WPC:(core + 1) * WPC, NCH:] = r["gout"]
    return out.reshape(B, S, NCH + WORD_EMB)
